# revision 20
# baseline (speedup 1.0000x reference)
"""Trainium2 Bass kernel for nn_CDAN_Dis (CDAN discriminator head).

Math per sample m (see reference):
  a    = einsum('cf,bft->bct', w2d, feature)            # [C,T]
  d    = einsum('bct,bcpt->bpt', a, mask) + b2d         # [P,T]
  d    = leaky(GLN_scalar(d))                           # global LN over (P,T)
  x1   = leaky(GLN_vec(conv1d(d,  w1,b1, s2,p1)))       # [256,1000]
  x2   = leaky(GLN_vec(conv1d(x1, w2,b2, s2,p1)))       # [256,500]
  out  = conv1d(x2, w3, b3, s1, p0)                     # [1,500]

Sharding: data-parallel over batch M=4 across 4 NeuronCores (one sample per
core).  Convs run as TensorE matmuls in bf16 (fast weight load); the mask
contraction runs on VectorE; GLN stats are fused into the elementwise passes
(accum_out) with row sums on ACT (Copy+accum) and square sums on DVE
(tensor_tensor_reduce); the cross-partition total + broadcast is a single
ones[128x128] matmul so every partition holds the totals and no second
broadcast hop is needed.

Perf-critical details vs the fp32 baseline (~48us/iter -> ~21-25us/iter):
 - all matmul weights/activations bf16 -> FWL weight loads, half DMA bytes
 - conv biases folded into the GLN stats + Prelu bias (saves 6 N=500 matmuls)
 - no PE warm-up matmuls (they only add PE busy cycles)
 - single flat GLN chain: stat-matmul -> reduce -> 2 tiny STT -> sqrt ->
   reciprocal -> scale/bias -> Prelu  (4 cross-engine hops)
 - software-pipelined emission across repeat iterations (front:
   DMA+stage1+GLN1 | mid: conv1+GLN2 | tail: conv2+GLN3+conv3): per-engine
   instruction streams are in-order, so without interleaving every GLN
   chain head-of-line-blocks the PE; pipelining takes PE to ~98% busy
 - output DMA via SWDGE (gpsimd) so it never blocks the next iteration's
   input DMAs on the sync HWDGE ring
 - b2d is a uniform additive constant immediately followed by a global
   layernorm, so it cancels exactly and is ignored.
"""

import sys

sys.path.insert(0, "/opt/trn_rl_repo")

from contextlib import ExitStack

import numpy as np

import concourse.bass as bass
import concourse.mybir as mybir
import concourse.tile as tile
from concourse import bacc, bass_utils

F32 = mybir.dt.float32
F32R = mybir.dt.float32r
BF16 = mybir.dt.bfloat16
AX = mybir.AxisListType
OP = mybir.AluOpType
AF = mybir.ActivationFunctionType

M, C, B, T = 4, 2, 128, 2000
TC = 500               # matmul free-dim chunk (PSUM bank limit)
NCHUNK = T // TC       # 4
T1 = 1000              # conv1 output length
T2 = 500               # conv2 output length
EPS = 1e-8

N1 = B * T             # GLN1 element count
N2 = 256 * T1
N3 = 256 * T2

USE_PRELU = True       # fused affine+leaky on ACT (Prelu alpha=0.1)
STOP_AFTER = "full"    # debug bisection: stage1|gln1|conv1|conv2|full
N_CORES = 4

# packed bf16 weights column offsets (CWB: [128, CWBW] bf16)
CW_W2DR = 0
CW_W1T = 256
CW_W2T = 1024
CW_W3T = 2560
CWBW = 2692   # pad past CW_W3T+129 for the conv3 128-col lhsT reads

# packed f32 constants column offsets (CF: [128, CFW] f32)
CF_ONES = 0            # [128,128] all ones (stat reduce+broadcast lhsT)
CF_EPS = 128
CF_G2D = 129
CF_BE2D = 130
CF_B1DUP = 131         # [128,4]  b1 dup per (oh,tcb) group
CF_B1R = 135           # [128,2]  b1 per oh
CF_G1 = 137
CF_BB1 = 139
CF_B2R = 141           # [128,2]  b2 per oh (dup == itself for conv2 groups)
CF_G2 = 143
CF_BB2 = 145
CF_B1K = 147           # [128,2]  1000*b1 per oh
CF_B2K = 149           # [128,2]  500*b2 per oh
CFW = 152
# packed-rows offsets (RW: [1, RWW] f32)
RW_B3 = 0
RWW = 16


def R(ap):
    return ap.bitcast(F32R)


def _patch_act_tables():
    """Pin every ACT func we use to the one set that has them all."""
    if getattr(bacc, "_cdan_act_patch", False):
        return
    orig = bacc.get_activation_tables
    mine = {AF.Copy, AF.Identity, AF.Square, AF.Sqrt, AF.Prelu}

    def patched(arch):
        t = dict(orig(arch))
        for name in t:
            if name != "sqrt_and_others":
                t[name] = set(t[name]) - mine
        return t

    bacc.get_activation_tables = patched
    bacc._cdan_act_patch = True


def build_nc(repeat=1):
    _patch_act_tables()
    nc = bacc.Bacc("TRN2", target_bir_lowering=False, debug=False,
                   num_devices=N_CORES)

    featb_d = nc.dram_tensor("featb", [B, T], BF16, kind="ExternalInput").ap()
    maskb_d = nc.dram_tensor("maskb", [B, C * T], BF16,
                             kind="ExternalInput").ap()
    cwb_d = nc.dram_tensor("cwb", [128, CWBW], BF16, kind="ExternalInput").ap()
    cf_d = nc.dram_tensor("cf", [128, CFW], F32, kind="ExternalInput").ap()
    rw_d = nc.dram_tensor("rw", [1, RWW], F32, kind="ExternalInput").ap()
    out_d = nc.dram_tensor("out", [1, T2], F32, kind="ExternalOutput").ap()

    with tile.TileContext(nc) as tc:
        with ExitStack() as ctx:
            pools = _make_pools(ctx, tc)
            # software pipeline: emit iteration i+1's front (DMA+stage1+GLN1)
            # before iteration i's back (convs) so every engine stream can
            # fill the GLN serialization stalls with the next iteration.
            q = []
            for _ in range(repeat):
                q.append(_emit_front(pools, tc, featb_d, maskb_d, cwb_d,
                                     cf_d, rw_d, out_d))
                if len(q) >= 2:
                    _emit_mid(pools, tc, q[-2])
                if len(q) >= 3:
                    _emit_tail(pools, tc, q[-3])
            if len(q) >= 2:
                _emit_mid(pools, tc, q[-1])
                _emit_tail(pools, tc, q[-2])
                _emit_tail(pools, tc, q[-1])
            elif q:
                _emit_mid(pools, tc, q[-1])
                _emit_tail(pools, tc, q[-1])
    nc.compile()
    return nc


def _make_pools(ctx, tc):
    class P:
        pass
    p = P()
    p.const = ctx.enter_context(tc.tile_pool(name="const", bufs=4))
    p.inp = ctx.enter_context(tc.tile_pool(name="inp", bufs=3))
    p.tmpp = ctx.enter_context(tc.tile_pool(name="tmpp", bufs=5))
    p.sqp = ctx.enter_context(tc.tile_pool(name="sqp", bufs=3))
    p.bigp = ctx.enter_context(tc.tile_pool(name="bigp", bufs=3))
    p.smallp = ctx.enter_context(tc.tile_pool(name="smallp", bufs=2))
    p.ps2 = ctx.enter_context(tc.tile_pool(name="ps2", bufs=3, space="PSUM"))
    p.ps1 = ctx.enter_context(tc.tile_pool(name="ps1", bufs=2, space="PSUM"))
    return p


def _gln_chain(nc, pool, invn, s12, eps_ap, tag):
    """From s12=[128,2]=(S1,S2) totals (every partition) produce
    (negmean, rstd) as [128,1] SBUF tiles."""
    tA = pool.tile([128, 1], F32, tag=f"tA{tag}")
    nc.vector.scalar_tensor_tensor(tA[:], s12[:, 0:1], invn, s12[:, 0:1],
                                   OP.mult, OP.mult)          # S1^2/N
    u = pool.tile([128, 1], F32, tag=f"u{tag}")
    nc.vector.scalar_tensor_tensor(u[:], tA[:], -1.0, s12[:, 1:2],
                                   OP.mult, OP.add)           # S2 - S1^2/N
    nm = pool.tile([128, 1], F32, tag=f"nm{tag}")
    nc.vector.tensor_scalar_mul(nm[:], s12[:, 0:1], -invn)    # -mean
    sstd = pool.tile([128, 1], F32, tag=f"sstd{tag}")
    nc.scalar.activation(sstd[:], u[:], AF.Sqrt, bias=eps_ap, scale=invn)
    rstd = pool.tile([128, 1], F32, tag=f"rstd{tag}")
    nc.vector.reciprocal(rstd[:], sstd[:])
    return nm, rstd


def _norm_leaky(nc, tmpp, out_ap, in_ap, scale_ap, bias_ap):
    """out = leaky(in*scale + bias), slope 0.1."""
    if USE_PRELU:
        nc.scalar.activation(out_ap, in_ap, AF.Prelu,
                             bias=bias_ap, scale=scale_ap, alpha=0.1)
    else:
        nfree = 1
        for s in out_ap.shape[1:]:
            nfree *= s
        af = tmpp.tile([128, nfree], F32, tag="t")
        nc.scalar.activation(af[:], in_ap, AF.Identity,
                             bias=bias_ap, scale=scale_ap)
        nc.vector.scalar_tensor_tensor(out_ap, af[:], 0.1, af[:],
                                       OP.mult, OP.max)


def _emit_front(pools, tc, featb_d, maskb_d, cwb_d, cf_d, rw_d, out_d):
    nc = tc.nc
    const, inp, tmpp = pools.const, pools.inp, pools.tmpp
    sqp, bigp, smallp = pools.sqp, pools.bigp, pools.smallp
    ps2, ps1 = pools.ps2, pools.ps1

    # ---- batched input DMAs (inputs on SP/ACT HWDGE; output goes SWDGE) ----
    feat = inp.tile([128, T], BF16, tag="feat")
    nc.sync.dma_start(feat[:], featb_d[:])
    cwb = const.tile([128, CWBW], BF16, tag="cwb")
    nc.scalar.dma_start(cwb[:, 0:256], cwb_d[:, 0:256])        # w2dr early
    mk = inp.tile([128, C * T], BF16, tag="mk")
    nc.sync.dma_start(mk[:], maskb_d[:])
    cf = const.tile([128, CFW], F32, tag="cf")
    nc.scalar.dma_start(cf[:], cf_d[:])
    rwt = const.tile([1, RWW], F32, tag="rw")
    nc.scalar.dma_start(rwt[:], rw_d[:])
    nc.scalar.dma_start(cwb[:, 256:CWBW], cwb_d[:, 256:CWBW])

    # views into the packs
    w2dr = cwb[:, CW_W2DR:CW_W2DR + 256]
    w1t = cwb[:, CW_W1T:CW_W1T + 768]
    w2t = cwb[:, CW_W2T:CW_W2T + 1536]
    onesf = cf[:, CF_ONES:CF_ONES + 128]
    eps_ap = cf[:, CF_EPS:CF_EPS + 1]
    g2d_c = cf[:, CF_G2D:CF_G2D + 1]
    be2d_c = cf[:, CF_BE2D:CF_BE2D + 1]
    b1dup = cf[:, CF_B1DUP:CF_B1DUP + 4]
    b1r = cf[:, CF_B1R:CF_B1R + 2]
    g1r = cf[:, CF_G1:CF_G1 + 2]
    bb1r = cf[:, CF_BB1:CF_BB1 + 2]
    b2r = cf[:, CF_B2R:CF_B2R + 2]
    g2r = cf[:, CF_G2:CF_G2 + 2]
    bb2r = cf[:, CF_BB2:CF_BB2 + 2]
    b1k = cf[:, CF_B1K:CF_B1K + 2]
    b2k = cf[:, CF_B2K:CF_B2K + 2]
    b3_ap = rwt[0:1, RW_B3:RW_B3 + 1]

    mk3 = mk[:].rearrange("p (c t) -> p c t", c=2)

    d = bigp.tile([128, T], F32, tag="d")
    st1 = smallp.tile([128, 2 * NCHUNK], F32, tag="st1")

    # ---- stage 1: d = mask0*bcast(a0) + mask1*bcast(a1), fused stats ----
    for j in range(NCHUNK):
        sl = slice(j * TC, (j + 1) * TC)
        a01 = ps2.tile([128, 1024], F32, tag="mm2")
        nc.tensor.matmul(a01[:, 0:TC], w2dr[:, 0:128], feat[:, sl],
                         start=True, stop=True)
        nc.tensor.matmul(a01[:, 512:512 + TC], w2dr[:, 128:256], feat[:, sl],
                         start=True, stop=True)
        a3 = a01[:].rearrange("p (c t) -> p c t", c=2)[:, :, 0:TC]
        t01 = tmpp.tile([128, 2 * TC], F32, tag="t01")
        nc.vector.tensor_mul(t01[:].rearrange("p (c t) -> p c t", c=2),
                             mk3[:, :, sl], a3)
        nc.vector.scalar_tensor_tensor(d[:, sl], t01[:, 0:TC], 0.0,
                                       t01[:, TC:2 * TC], OP.add, OP.add,
                                       accum_out=st1[:, j:j + 1])
        sq = sqp.tile([128, TC], F32, tag="sq")
        nc.scalar.activation(sq[:], d[:, sl], AF.Square,
                             accum_out=st1[:, NCHUNK + j:NCHUNK + j + 1])

    if STOP_AFTER == "stage1":
        out_s = smallp.tile([1, T2], F32, tag="out_s")
        nc.vector.tensor_copy(out_s[:], d[0:1, 0:T2])
        nc.gpsimd.dma_start(out_d[:], out_s[:])
        return None

    # ---- GLN1: totals on every partition via ones-matmul, flat chain ----
    psS = ps1.tile([128, 512], F32, tag="mm1")
    nc.tensor.matmul(psS[:, 0:2 * NCHUNK], onesf, st1[:],
                     start=True, stop=True)
    s12 = smallp.tile([128, 2], F32, tag="s12_1")
    nc.vector.reduce_sum(s12[:],
                         psS[:, 0:2 * NCHUNK].rearrange("p (a b) -> p a b", a=2),
                         axis=AX.X)
    nm1, rstd1 = _gln_chain(nc, smallp, 1.0 / N1, s12, eps_ap, "1")
    scl1 = smallp.tile([128, 1], F32, tag="scl1")
    nc.vector.tensor_mul(scl1[:], rstd1[:], g2d_c)
    bia1 = smallp.tile([128, 1], F32, tag="bia1")
    nc.vector.scalar_tensor_tensor(bia1[:], scl1[:], nm1[:], be2d_c,
                                   OP.mult, OP.add)

    # ---- GLN1 normalize + leaky -> xpad (bf16), one wide pass ----
    xpad = bigp.tile([128, T + 2], BF16, tag="xpad")
    nc.vector.tensor_scalar_mul(xpad[:, 0:1], onesf[:, 0:1], 0.0)
    nc.vector.tensor_scalar_mul(xpad[:, T + 1:T + 2], onesf[:, 0:1], 0.0)
    _norm_leaky(nc, tmpp, xpad[:, 1:T + 1], d[:, 0:T], scl1[:], bia1[:])

    if STOP_AFTER == "gln1":
        out_s = smallp.tile([1, T2], F32, tag="out_s")
        nc.vector.tensor_copy(out_s[:], xpad[0:1, 1:T2 + 1])
        nc.gpsimd.dma_start(out_d[:], out_s[:])
        return None

    return dict(cwb=cwb, cf=cf, rwt=rwt, xpad=xpad,
                w1t=w1t, w2t=w2t, onesf=onesf, eps_ap=eps_ap,
                b1dup=b1dup, b1r=b1r, g1r=g1r, bb1r=bb1r,
                b2r=b2r, g2r=g2r, bb2r=bb2r, b1k=b1k, b2k=b2k, b3_ap=b3_ap,
                out_d=out_d)


def _emit_mid(pools, tc, st):
    if st is None:
        return
    nc = tc.nc
    const, inp, tmpp = pools.const, pools.inp, pools.tmpp
    sqp, bigp, smallp = pools.sqp, pools.bigp, pools.smallp
    ps2, ps1 = pools.ps2, pools.ps1
    cwb = st["cwb"]
    xpad = st["xpad"]
    w1t, w2t = st["w1t"], st["w2t"]
    onesf, eps_ap = st["onesf"], st["eps_ap"]
    b1dup, b1r, g1r, bb1r = st["b1dup"], st["b1r"], st["g1r"], st["bb1r"]
    b2r, g2r, bb2r = st["b2r"], st["g2r"], st["bb2r"]
    b1k, b2k, b3_ap = st["b1k"], st["b2k"], st["b3_ap"]
    out_d = st["out_d"]


    # ---- conv1 (128->256, k3 s2 p1): one 2-bank psum tile per oh ----
    # st2: cols 0-3 raw S1 per (oh,tcb); cols 4-5 raw S2 per oh
    st2 = smallp.tile([128, 8], F32, tag="st2")
    py1 = {}
    for oh in range(2):
        p = ps2.tile([128, 1024], F32, tag="mm2")
        py1[oh] = p
        for k in range(3):
            for tcb in range(2):
                rhs = xpad[:, k + 2 * (tcb * T2): k + 2 * (tcb * T2) + 2 * T2 - 1:2]
                nc.tensor.matmul(p[:, tcb * 512:tcb * 512 + T2],
                                 w1t[:, k * 256 + oh * 128:
                                     k * 256 + oh * 128 + 128],
                                 rhs, start=(k == 0), stop=(k == 2))
        p3v = p[:].rearrange("p (a b) -> p a b", a=2)[:, :, 0:T2]
        nc.vector.reduce_sum(st2[:, 2 * oh:2 * oh + 2], p3v, axis=AX.X)
        sq = sqp.tile([128, 2 * TC], F32, tag="sq2")
        nc.scalar.activation(sq[:].rearrange("p (a b) -> p a b", a=2), p3v,
                             AF.Square, accum_out=st2[:, 4 + oh:5 + oh])

    # ---- fold b1 into the raw stats ----
    # S1'[oh,tcb] = S1 + 500 b    (4 groups of 500)
    # S2'[oh] = S2 + b*(2*S1tot_oh + 1000 b)
    st2adj = smallp.tile([128, 8], F32, tag="st2adj")
    nc.vector.scalar_tensor_tensor(st2adj[:, 0:4], b1dup, 500.0, st2[:, 0:4],
                                   OP.mult, OP.add)
    s1oh = smallp.tile([128, 2], F32, tag="s1oh")
    nc.vector.scalar_tensor_tensor(s1oh[:], st2[:, 0:4:2], 1.0,
                                   st2[:, 1:4:2], OP.mult, OP.add)
    tb = smallp.tile([128, 2], F32, tag="tb2")
    nc.vector.scalar_tensor_tensor(tb[:], s1oh[:], 2.0, b1k, OP.mult, OP.add)
    tb2 = smallp.tile([128, 2], F32, tag="tb2b")
    nc.vector.tensor_mul(tb2[:], tb[:], b1r)
    nc.vector.tensor_add(st2adj[:, 4:6], tb2[:], st2[:, 4:6])

    # ---- GLN2 chain ----
    psS2 = ps1.tile([128, 512], F32, tag="mm1")
    nc.tensor.matmul(psS2[:, 0:6], onesf, st2adj[:, 0:6],
                     start=True, stop=True)
    s12_2 = smallp.tile([128, 2], F32, tag="s12_2")
    nc.vector.reduce_sum(s12_2[:, 0:1], psS2[:, 0:4], axis=AX.X)
    nc.vector.reduce_sum(s12_2[:, 1:2], psS2[:, 4:6], axis=AX.X)
    nm2, rstd2 = _gln_chain(nc, smallp, 1.0 / N2, s12_2, eps_ap, "2")
    scl2 = smallp.tile([128, 2], F32, tag="scl2")
    nc.vector.tensor_scalar_mul(scl2[:], g1r, rstd2[:])
    bia2 = smallp.tile([128, 2], F32, tag="bia2")
    nc.vector.scalar_tensor_tensor(bia2[:], scl2[:], nm2[:], bb1r,
                                   OP.mult, OP.add)
    tb3 = smallp.tile([128, 2], F32, tag="tb3")
    nc.vector.tensor_mul(tb3[:], scl2[:], b1r)
    bia2f = smallp.tile([128, 2], F32, tag="bia2f")
    nc.vector.tensor_add(bia2f[:], tb3[:], bia2[:])

    # ---- GLN2 normalize + leaky -> y1pad (bf16), one wide pass per oh ----
    y1pad = []
    for oh in range(2):
        yp = bigp.tile([128, T1 + 2], BF16, tag=f"y1pad{oh}")
        y1pad.append(yp)
        nc.vector.tensor_scalar_mul(yp[:, 0:1], onesf[:, 0:1], 0.0)
        nc.vector.tensor_scalar_mul(yp[:, T1 + 1:T1 + 2], onesf[:, 0:1], 0.0)
        pin = py1[oh][:].rearrange("p (a b) -> p a b", a=2)[:, :, 0:T2]
        pout = yp[:, 1:T1 + 1].rearrange("p (a b) -> p a b", a=2)
        _norm_leaky(nc, tmpp, pout, pin,
                    scl2[:, oh:oh + 1], bia2f[:, oh:oh + 1])

    if STOP_AFTER == "conv1":
        out_s = smallp.tile([1, T2], F32, tag="out_s")
        nc.vector.tensor_copy(out_s[:], y1pad[0][0:1, 1:T2 + 1])
        nc.gpsimd.dma_start(out_d[:], out_s[:])
        return

    st["y1pad"] = y1pad


def _emit_tail(pools, tc, st):
    if st is None:
        return
    nc = tc.nc
    const, inp, tmpp = pools.const, pools.inp, pools.tmpp
    sqp, bigp, smallp = pools.sqp, pools.bigp, pools.smallp
    ps2, ps1 = pools.ps2, pools.ps1
    cwb = st["cwb"]
    w2t = st["w2t"]
    onesf, eps_ap = st["onesf"], st["eps_ap"]
    b2r, g2r, bb2r = st["b2r"], st["g2r"], st["bb2r"]
    b2k, b3_ap = st["b2k"], st["b3_ap"]
    out_d = st["out_d"]
    y1pad = st["y1pad"]


    # ---- conv2 (256->256, k3 s2 p1): one 2-bank tile, oh halves ----
    st3 = smallp.tile([128, 8], F32, tag="st3")
    p2 = ps2.tile([128, 1024], F32, tag="mm2")
    for oh in range(2):
        first = True
        for cih in range(2):
            for k in range(3):
                rhs = y1pad[cih][:, k: k + 2 * T2 - 1:2]
                nc.tensor.matmul(p2[:, oh * 512:oh * 512 + T2],
                                 w2t[:, cih * 768 + k * 256 + oh * 128:
                                     cih * 768 + k * 256 + oh * 128 + 128],
                                 rhs, start=first,
                                 stop=(cih == 1 and k == 2))
                first = False
    p2v = p2[:].rearrange("p (a b) -> p a b", a=2)[:, :, 0:T2]
    nc.vector.reduce_sum(st3[:, 0:2], p2v, axis=AX.X)
    for oh in range(2):
        sq = sqp.tile([128, TC], F32, tag="sq")
        nc.scalar.activation(sq[:], p2[:, oh * 512:oh * 512 + T2], AF.Square,
                             accum_out=st3[:, 2 + oh:3 + oh])

    # ---- fold b2 ----
    st3adj = smallp.tile([128, 4], F32, tag="st3adj")
    nc.vector.scalar_tensor_tensor(st3adj[:, 0:2], b2r, 500.0, st3[:, 0:2],
                                   OP.mult, OP.add)
    tc1 = smallp.tile([128, 2], F32, tag="tc1")
    nc.vector.scalar_tensor_tensor(tc1[:], st3[:, 0:2], 2.0, b2k,
                                   OP.mult, OP.add)
    tc2 = smallp.tile([128, 2], F32, tag="tc2")
    nc.vector.tensor_mul(tc2[:], tc1[:], b2r)
    nc.vector.tensor_add(st3adj[:, 2:4], tc2[:], st3[:, 2:4])

    # ---- GLN3 chain ----
    psS3 = ps1.tile([128, 512], F32, tag="mm1")
    nc.tensor.matmul(psS3[:, 0:4], onesf, st3adj[:],
                     start=True, stop=True)
    s12_3 = smallp.tile([128, 2], F32, tag="s12_3")
    nc.vector.reduce_sum(s12_3[:],
                         psS3[:, 0:4].rearrange("p (a b) -> p a b", a=2),
                         axis=AX.X)
    nm3, rstd3 = _gln_chain(nc, smallp, 1.0 / N3, s12_3, eps_ap, "3")
    scl3 = smallp.tile([128, 2], F32, tag="scl3")
    nc.vector.tensor_scalar_mul(scl3[:], g2r, rstd3[:])
    bia3 = smallp.tile([128, 2], F32, tag="bia3")
    nc.vector.scalar_tensor_tensor(bia3[:], scl3[:], nm3[:], bb2r,
                                   OP.mult, OP.add)
    tc3 = smallp.tile([128, 2], F32, tag="tc3")
    nc.vector.tensor_mul(tc3[:], scl3[:], b2r)
    bia3f = smallp.tile([128, 2], F32, tag="bia3f")
    nc.vector.tensor_add(bia3f[:], tc3[:], bia3[:])

    # ---- GLN3 normalize + leaky -> x3 halves (bf16) ----
    x3 = []
    for oh in range(2):
        xt = bigp.tile([128, T2], BF16, tag=f"x3_{oh}")
        x3.append(xt)
        _norm_leaky(nc, tmpp, xt[:], p2[:, oh * 512:oh * 512 + T2],
                    scl3[:, oh:oh + 1], bia3f[:, oh:oh + 1])

    if STOP_AFTER == "conv2":
        out_s = smallp.tile([1, T2], F32, tag="out_s")
        nc.vector.tensor_copy(out_s[:], x3[0][0:1, :])
        nc.gpsimd.dma_start(out_d[:], out_s[:])
        return

    # ---- conv3 (256->1, k1) + b3 ----
    # bf16 needs M=128: lhsT is 128 consecutive CWB columns whose col0 holds
    # w3 for the half; rows 1..127 of the psum accumulate garbage that we
    # never read.
    p3 = ps1.tile([128, 512], F32, tag="mm1")
    nc.tensor.matmul(p3[:, 0:T2], cwb[:, CW_W3T:CW_W3T + 128], x3[0][:],
                     start=True, stop=False)
    nc.tensor.matmul(p3[:, 0:T2], cwb[:, CW_W3T + 1:CW_W3T + 129], x3[1][:],
                     start=False, stop=True)
    out_s = smallp.tile([1, T2], F32, tag="out_s")
    nc.scalar.activation(out_s[:], p3[0:1, 0:T2], AF.Identity,
                         bias=b3_ap, scale=1.0)
    nc.gpsimd.dma_start(out_d[:], out_s[:])


def shard_inputs(inputs):
    """Full inputs -> per-core in_maps (host-side layout prep)."""
    import ml_dtypes
    bf = ml_dtypes.bfloat16
    f = {k: np.ascontiguousarray(np.asarray(v, dtype=np.float32))
         for k, v in inputs.items()}

    cwb = np.zeros((128, CWBW), np.float32)
    w2d = f["w2d"]
    cwb[:, CW_W2DR:CW_W2DR + 128] = np.tile(w2d[0][:, None], (1, 128))
    cwb[:, CW_W2DR + 128:CW_W2DR + 256] = np.tile(w2d[1][:, None], (1, 128))
    cwb[:, CW_W1T:CW_W1T + 768] = f["w1"].transpose(1, 2, 0).reshape(128, 768)
    cwb[:, CW_W2T:CW_W2T + 1536] = (
        f["w2"].transpose(1, 2, 0).reshape(2, 128, 3, 256)
        .transpose(1, 0, 2, 3).reshape(128, 1536))
    cwb[:, CW_W3T:CW_W3T + 2] = f["w3"].reshape(2, 128).T
    cwb = cwb.astype(bf)

    cf = np.zeros((128, CFW), np.float32)
    cf[:, CF_ONES:CF_ONES + 128] = 1.0
    cf[:, CF_EPS] = EPS
    cf[:, CF_G2D] = float(f["g2d"].reshape(()))
    cf[:, CF_BE2D] = float(f["be2d"].reshape(()))
    b1 = f["b1"].reshape(2, 128).T          # [128, 2] per oh
    cf[:, CF_B1DUP + 0] = b1[:, 0]
    cf[:, CF_B1DUP + 1] = b1[:, 0]
    cf[:, CF_B1DUP + 2] = b1[:, 1]
    cf[:, CF_B1DUP + 3] = b1[:, 1]
    cf[:, CF_B1R:CF_B1R + 2] = b1
    cf[:, CF_G1:CF_G1 + 2] = f["g1"].reshape(2, 128).T
    cf[:, CF_BB1:CF_BB1 + 2] = f["bb1"].reshape(2, 128).T
    b2 = f["b2"].reshape(2, 128).T
    cf[:, CF_B2R:CF_B2R + 2] = b2
    cf[:, CF_G2:CF_G2 + 2] = f["g2"].reshape(2, 128).T
    cf[:, CF_BB2:CF_BB2 + 2] = f["bb2"].reshape(2, 128).T
    cf[:, CF_B1K:CF_B1K + 2] = 1000.0 * b1
    cf[:, CF_B2K:CF_B2K + 2] = 500.0 * b2

    rw = np.zeros((1, RWW), np.float32)
    rw[0, RW_B3] = float(f["b3"].reshape(()))

    featb = f["feature"].astype(bf)                     # [M,128,T]
    maskb = f["mask"].transpose(0, 2, 1, 3).reshape(M, 128, C * T).astype(bf)

    in_maps = []
    for i in range(M):
        in_maps.append(dict(cwb=cwb, cf=cf, rw=rw,
                            featb=np.ascontiguousarray(featb[i]),
                            maskb=np.ascontiguousarray(maskb[i])))
    return in_maps


_NC = None


def kernel(**inputs):
    global _NC
    if _NC is None:
        _NC = build_nc()
    in_maps = shard_inputs(inputs)
    res = bass_utils.run_bass_kernel_spmd(_NC, in_maps,
                                          core_ids=list(range(N_CORES)))
    out = np.stack([res.results[i]["out"] for i in range(M)], axis=0)
    return out.astype(np.float32)
